# revision 1
# baseline (speedup 1.0000x reference)
"""Distributed Trainium2 (8 NeuronCores) attention-head kernel.

Problem: single attention head with projections.
  q = Q @ Wq.T + bq ; k = K @ Wk.T + bk ; v = V @ Wv.T + bv
  x = (q @ k.T) / sqrt(64) ; x = x*m - 1e9*(1-m) ; p = softmax(x)
  y = p @ v
Shapes: Q/K/V [2, 4096, 1024] f32, mask [2, 4096, 4096] int32 -> y [2, 4096, 64] f32.

Strategy (8 cores): shard queries 8-ways (2 batches x 4 query-chunks of 1024
rows).  K/V are replicated within each 4-core batch group (collective_compute
has ~100us fixed overhead on this fleet; bf16 replication is cheaper).  The
host reshards into matmul-native transposed layouts (contraction dim on
partitions), packed partition-major so every DMA is a full-width [128, W] tile
with >=8KB contiguous per-partition rows (measured ~430GB/s vs 41GB/s at 1KB
rows).  K/V are additionally packed s-group-major so each 2MB group load is
immediately projectable (the dm-contraction needs all 8 dm-chunks of a group).
Q/K/V/W are cast to bf16, the 0/1 mask to fp8e4 (exact); softmax is computed
as p=exp(x/8)*m, y=(p@v)/sum(p) - algebraically identical to the reference's
masked softmax (no fully-masked rows exist).

Per-core pipeline (layouts [partitions, free]):
  qT[64,1024] = sum_j WqT[j].T @ QT[j]        (dm-chunk j, PSUM accumulate)
  per s-group g (4 x 1024): kT[:,g] = proj(K), vT[:,g] = proj(V),
     v_aug[s,65] = [v | 1] via PE transposes of vT
     per s-chunk j (8 x 128): sT = kT_chunk.T @ qT ; p = exp(sT/8) * maskT
                              yT[65,1024] += v_aug_chunk.T @ p  (PSUM accum)
  y[q,65] = transpose(yT); out = y[:, :64] / y[:, 64:65]
DMA issue order == consumption order (per-engine FIFO queues preserve it).
"""

import numpy as np
import ml_dtypes

import concourse.bass as bass
import concourse.mybir as mybir
import concourse.tile as tile
from concourse import bacc
import concourse.bass_utils as bass_utils
from concourse.bass_utils import run_bass_kernel_spmd
from concourse.masks import make_identity

B, S, DM, DK = 2, 4096, 1024, 64
N_CORES = 8
GROUP = 4            # cores per batch
SQ = S // GROUP      # query rows per core (1024)
NDM = DM // 128      # dm chunks (8)
NSG = 4              # s groups (1024 rows each)
SG = S // NSG        # 1024
NSC_G = SG // 128    # s chunks per group (8)

F32 = mybir.dt.float32
BF16 = mybir.dt.bfloat16
FP8 = mybir.dt.float8e4

_last_results = None


def _build():
    nc = bacc.Bacc(None, target_bir_lowering=False)

    qt_e = nc.declare_dram_parameter("qt", [128, NDM * SQ], BF16, isOutput=False)
    kt_e = nc.declare_dram_parameter("kt", [128, NDM * S], BF16, isOutput=False)
    vt_e = nc.declare_dram_parameter("vt", [128, NDM * S], BF16, isOutput=False)
    mt_e = nc.declare_dram_parameter("mt", [128, S * SQ // 128], FP8, isOutput=False)
    w_e = nc.declare_dram_parameter("wqkv", [128, 3 * NDM * DK], BF16, isOutput=False)
    b_e = nc.declare_dram_parameter("bqkv", [DK, 3], F32, isOutput=False)
    out_e = nc.declare_dram_parameter("out", [128, (SQ // 128) * DK], F32, isOutput=True)

    GW = NDM * SG    # columns per kt/vt group slice (8192)
    MW = NSC_G * SQ  # columns per mask group slice (8192)

    with tile.TileContext(nc) as tc:
        with (
            tc.tile_pool(name="const", bufs=1) as cpool,
            tc.tile_pool(name="mask", bufs=NSG) as mpool,
            tc.tile_pool(name="kin", bufs=3) as kpool,
            tc.tile_pool(name="big", bufs=1) as bigpool,
            tc.tile_pool(name="pp", bufs=6) as ppool,
            tc.tile_pool(name="small", bufs=4) as spool,
            tc.tile_pool(name="psum", bufs=1, space="PSUM") as pproj,
            tc.tile_pool(name="psum_s", bufs=2, space="PSUM") as psT,
            tc.tile_pool(name="psum_y", bufs=1, space="PSUM") as pyT,
        ):
            # ---- loads, in consumption order ----
            qt_sb = cpool.tile([128, NDM * SQ], BF16, tag="qt")
            nc.sync.dma_start(qt_sb[:], qt_e[:])
            w_sb = cpool.tile([128, 3 * NDM * DK], BF16, tag="w")
            nc.sync.dma_start(w_sb[:], w_e[:])
            b_sb = cpool.tile([DK, 3], F32, tag="b")
            nc.sync.dma_start(b_sb[:], b_e[:])

            kt_t, vt_t, mq_t = {}, {}, {}
            for g in range(NSG):
                kt_t[g] = kpool.tile([128, GW], BF16, tag="xin", name=f"kt_g{g}")
                nc.sync.dma_start(kt_t[g][:], kt_e[:, g * GW:(g + 1) * GW])
                if g == 0:
                    mq_t[0] = mpool.tile([128, MW], FP8, tag="mt", name="mq_g0")
                    nc.sync.dma_start(mq_t[0][:], mt_e[:, 0:MW])
                vt_t[g] = kpool.tile([128, GW], BF16, tag="xin", name=f"vt_g{g}")
                nc.sync.dma_start(vt_t[g][:], vt_e[:, g * GW:(g + 1) * GW])
                if g in (1, 2):
                    mq_t[g] = mpool.tile([128, MW], FP8, tag="mt", name=f"mq_g{g}")
                    nc.sync.dma_start(mq_t[g][:], mt_e[:, g * MW:(g + 1) * MW])
            mq_t[3] = mpool.tile([128, MW], FP8, tag="mt", name="mq_g3")
            nc.sync.dma_start(mq_t[3][:], mt_e[:, 3 * MW:4 * MW])

            ident_bf = cpool.tile([128, 128], BF16, tag="ident_bf")
            make_identity(nc, ident_bf[:])
            ident_f32 = cpool.tile([128, 128], F32, tag="ident_f32")
            make_identity(nc, ident_f32[:])

            def wsl(which, j):  # weight chunk slice in w_sb
                return w_sb[:, (which * NDM + j) * DK:(which * NDM + j + 1) * DK]

            # ---- q projection: qT[64, 1024] ----
            qT_sb = bigpool.tile([DK, SQ], BF16, tag="qT")
            ps = pproj.tile([DK, 1024], F32, tag="proj")
            for j in range(NDM):
                for h in range(2):
                    c0 = j * SQ + h * 512
                    nc.tensor.matmul(
                        ps[:, h * 512:(h + 1) * 512],
                        lhsT=wsl(0, j), rhs=qt_sb[:, c0:c0 + 512],
                        start=(j == 0), stop=(j == NDM - 1),
                    )
            nc.vector.tensor_scalar_add(qT_sb[:], ps[:], b_sb[:, 0:1])

            kT_sb = bigpool.tile([DK, S], BF16, tag="kT")
            vT_sb = bigpool.tile([DK, S], BF16, tag="vT")
            v_aug = bigpool.tile([128, S // 128 * 65], BF16, tag="vaug")
            nc.vector.memset(v_aug[:], 1.0)
            yT_ps = pyT.tile([65, SQ], F32, tag="yT")

            for g in range(NSG):
                # ---- k/v projections for this s-group ----
                for which, t, dst in ((1, kt_t[g], kT_sb), (2, vt_t[g], vT_sb)):
                    ps = pproj.tile([DK, 1024], F32, tag="proj", name=f"ps_{which}_{g}")
                    for j in range(NDM):
                        for h in range(2):
                            c0 = j * SG + h * 512
                            nc.tensor.matmul(
                                ps[:, h * 512:(h + 1) * 512],
                                lhsT=wsl(which, j), rhs=t[:, c0:c0 + 512],
                                start=(j == 0), stop=(j == NDM - 1),
                            )
                    nc.vector.tensor_scalar_add(
                        dst[:, g * SG:(g + 1) * SG], ps[:], b_sb[:, which:which + 1]
                    )
                # ---- v_aug transposes for this group ----
                for jj in range(NSC_G):
                    sc = g * NSC_G + jj
                    pt = psT.tile([128, DK], BF16, tag="sT", name=f"pt_{sc}")
                    nc.tensor.transpose(
                        pt[:], vT_sb[:, sc * 128:(sc + 1) * 128],
                        ident_bf[:DK, :DK],
                    )
                    nc.vector.tensor_copy(v_aug[:, sc * 65:sc * 65 + DK], pt[:])
                # ---- main loop chunks of this group ----
                for jj in range(NSC_G):
                    sc = g * NSC_G + jj
                    sT = psT.tile([128, SQ], F32, tag="sT", name=f"sT_{sc}")
                    for h in range(2):
                        nc.tensor.matmul(
                            sT[:, h * 512:(h + 1) * 512],
                            lhsT=kT_sb[:, sc * 128:(sc + 1) * 128],
                            rhs=qT_sb[:, h * 512:(h + 1) * 512],
                            start=True, stop=True,
                        )
                    p = ppool.tile([128, SQ], BF16, tag="p", name=f"p_{sc}")
                    nc.scalar.activation(
                        p[:], sT[:], mybir.ActivationFunctionType.Exp, scale=0.125
                    )
                    nc.vector.tensor_mul(
                        p[:], p[:], mq_t[g][:, jj * SQ:(jj + 1) * SQ]
                    )
                    for h in range(2):
                        nc.tensor.matmul(
                            yT_ps[:, h * 512:(h + 1) * 512],
                            lhsT=v_aug[:, sc * 65:(sc + 1) * 65],
                            rhs=p[:, h * 512:(h + 1) * 512],
                            start=(sc == 0), stop=(sc == S // 128 - 1),
                        )

            # ---- epilogue: y = transpose(yT); out = y[:, :64] / y[:, 64] ----
            yT_sb = bigpool.tile([65, SQ], F32, tag="yT_sb")
            nc.scalar.copy(yT_sb[:], yT_ps[:])
            y_all = bigpool.tile([128, (SQ // 128) * DK], F32, tag="y_all")
            for t in range(SQ // 128):
                yp = psT.tile([128, 65], F32, tag="sT", name=f"yp_{t}")
                nc.tensor.transpose(
                    yp[:], yT_sb[:, t * 128:(t + 1) * 128], ident_f32[:65, :65]
                )
                rcp = spool.tile([128, 1], F32, tag="rcp", name=f"rcp_{t}")
                nc.vector.reciprocal(rcp[:], yp[:, DK:DK + 1])
                nc.vector.tensor_scalar_mul(
                    y_all[:, t * DK:(t + 1) * DK], yp[:, :DK], rcp[:]
                )
            nc.sync.dma_start(out_e[:], y_all[:])

    nc.finalize()
    return nc


def _pack(at, w):
    """[R, W] -> [128, (R//128)*W]: row p gets rows {p, 128+p, ...}."""
    r = at.shape[0]
    return np.ascontiguousarray(
        at.reshape(r // 128, 128, w).transpose(1, 0, 2).reshape(128, -1)
    )


def _pack_groups(at):
    """KT/VT [1024, 4096] -> [128, 4*8*1024]: s-group-major partition packing.
    col ((g*8 + j)*1024 + s') on row p = at[j*128 + p, g*1024 + s']."""
    a = at.reshape(NDM, 128, NSG, SG)         # [j, p, g, s']
    return np.ascontiguousarray(
        a.transpose(1, 2, 0, 3).reshape(128, -1)
    )


def kernel(Q, K, V, mask, Wq, bq, Wk, bk, Wv, bv):
    global _last_results
    bf16 = ml_dtypes.bfloat16
    fp8 = ml_dtypes.float8_e4m3

    w_p = np.concatenate(
        [_pack(W.T.astype(bf16), DK) for W in (Wq, Wk, Wv)], axis=1
    )
    b_p = np.ascontiguousarray(
        np.stack([bq, bk, bv], axis=1).astype(np.float32)
    )

    kt_b = [_pack_groups(K[b].T.astype(bf16)) for b in range(B)]
    vt_b = [_pack_groups(V[b].T.astype(bf16)) for b in range(B)]

    in_maps = []
    for c in range(N_CORES):
        b, i = divmod(c, GROUP)
        rows = slice(i * SQ, (i + 1) * SQ)
        in_maps.append({
            "qt": _pack(np.ascontiguousarray(Q[b, rows, :].T).astype(bf16), SQ),
            "kt": kt_b[b],
            "vt": vt_b[b],
            "mt": _pack(np.ascontiguousarray(mask[b, rows, :].T).astype(fp8), SQ),
            "wqkv": w_p,
            "bqkv": b_p,
        })

    nc = _build()
    res = run_bass_kernel_spmd(nc, in_maps, core_ids=list(range(N_CORES)))
    _last_results = res

    out = np.empty((B, S, DK), dtype=np.float32)
    for c in range(N_CORES):
        b, i = divmod(c, GROUP)
        y = res.results[c]["out"].reshape(128, SQ // 128, DK).transpose(1, 0, 2)
        out[b, i * SQ:(i + 1) * SQ, :] = y.reshape(SQ, DK)
    return out



# revision 4
# speedup vs baseline: 1.2205x; 1.2205x over previous
"""Distributed Trainium2 (8 NeuronCores) attention-head kernel, key-sharded.

Problem: single attention head with projections.
  q = Q @ Wq.T + bq ; k = K @ Wk.T + bk ; v = V @ Wv.T + bv
  x = (q @ k.T) / sqrt(64) ; x = x*m - 1e9*(1-m) ; p = softmax(x)
  y = p @ v
Shapes: Q/K/V [2, 4096, 1024] f32, mask [2, 4096, 4096] int32 -> y [2, 4096, 64] f32.

Strategy (8 cores): shard KEYS 4-ways within each batch (core c -> batch c//4,
key rows (c%4)*1024..+1024).  Each core projects its 1024-key K/V slice (k/v
projection work is NOT duplicated, unlike query-sharding which replicates it
4x), projects all 4096 batch queries (q-proj is half the size of k+v), computes
partial attention p = exp(qk/8)*m over its key range, and returns the partial
numerator and denominator yT[65, 4096] = [v|1]^T @ p.  The host sums the 4
partials per batch and divides - algebraically identical to the reference's
masked softmax (no fully-masked rows exist).

Layouts ([partitions, free], contraction on partitions, all matmul N=512):
  head:   kT[64,1024] = sum_j WkT_j.T @ ktile_j   (PSUM, dm-chunk j)
          vT likewise; v_aug[128, 8*65] = [vT.T | 1] via PE transposes
  per qg (8 query-groups of 512):
          qT[64,512] = sum_j WqT_j.T @ qtile      (+bias on DVE)
          per kc (8 key-chunks of 128):
            sT[128,512] = kT_kc.T @ qT ; p = exp(sT/8) (Scalar) * mask (DVE/GpSimd)
            yT[65,512] += v_aug_kc.T @ p          (PSUM accum over kc)
          y_all[:, qg] = yT  (Scalar copy)
Issue order is software-pipelined (qproj of qg+1 before scores of qg) so the
PE instruction stream never waits on the exp->mask chain.  DMA issue order ==
consumption order (per-engine FIFO queues preserve it).
"""

import numpy as np
import ml_dtypes

import concourse.bass as bass
import concourse.mybir as mybir
import concourse.tile as tile
from concourse import bacc
import concourse.bass_utils as bass_utils
from concourse.bass_utils import run_bass_kernel_spmd
from concourse.masks import make_identity

B, S, DM, DK = 2, 4096, 1024, 64
N_CORES = 8
GROUP = 4              # cores per batch
SK = S // GROUP        # key rows per core (1024)
NDM = DM // 128        # dm chunks (8)
NQG = 8                # query groups
QG = S // NQG          # 512 queries per group
NKC = SK // 128        # key chunks per core (8)

F32 = mybir.dt.float32
BF16 = mybir.dt.bfloat16
FP8 = mybir.dt.float8e4

_last_results = None


def _build():
    nc = bacc.Bacc(None, target_bir_lowering=False)

    # kt/vt: [128, j(8) x 1024 keys] bf16; qt: [128, qg(8) x j(8) x 512] bf16
    # mt: [128, qg(8) x kc(8) x 512] fp8; w: [128, 3 x 8 x 64] bf16
    kt_e = nc.declare_dram_parameter("kt", [128, NDM * SK], BF16, isOutput=False)
    vt_e = nc.declare_dram_parameter("vt", [128, NDM * SK], BF16, isOutput=False)
    qt_e = nc.declare_dram_parameter("qt", [128, NQG * NDM * QG], BF16, isOutput=False)
    mt_e = nc.declare_dram_parameter("mt", [128, NQG * NKC * QG], FP8, isOutput=False)
    w_e = nc.declare_dram_parameter("wqkv", [128, 3 * NDM * DK], BF16, isOutput=False)
    b_e = nc.declare_dram_parameter("bqkv", [DK, 3], F32, isOutput=False)
    out_e = nc.declare_dram_parameter("out", [DK + 1, S], F32, isOutput=True)

    with tile.TileContext(nc) as tc:
        with (
            tc.tile_pool(name="const", bufs=1) as cpool,
            tc.tile_pool(name="kvin", bufs=8) as kvpool,
            tc.tile_pool(name="qin", bufs=3) as qpool,
            tc.tile_pool(name="min", bufs=3) as mpool,
            tc.tile_pool(name="qT", bufs=3) as qtpool,
            tc.tile_pool(name="p", bufs=8) as ppool,
            tc.tile_pool(name="psum_pr", bufs=2, space="PSUM") as pproj,
            tc.tile_pool(name="psum_s", bufs=4, space="PSUM") as psT,
            tc.tile_pool(name="psum_y", bufs=2, space="PSUM") as pyT,
        ):
            # ---- DMA loads, in consumption order ----
            w_sb = cpool.tile([128, 3 * NDM * DK], BF16, tag="w")
            nc.sync.dma_start(w_sb[:], w_e[:])
            b_sb = cpool.tile([DK, 3], F32, tag="b")
            nc.sync.dma_start(b_sb[:], b_e[:])

            kt_t, vt_t = [], []
            for jp in range(4):
                t = kvpool.tile([128, 2 * SK], BF16, tag="kv", name=f"kt_{jp}")
                nc.sync.dma_start(t[:], kt_e[:, jp * 2 * SK:(jp + 1) * 2 * SK])
                kt_t.append(t)
            for jp in range(4):
                t = kvpool.tile([128, 2 * SK], BF16, tag="kv", name=f"vt_{jp}")
                nc.sync.dma_start(t[:], vt_e[:, jp * 2 * SK:(jp + 1) * 2 * SK])
                vt_t.append(t)

            qt_t, mq_t = {}, {}
            QW = NDM * QG   # qt cols per qg (4096)
            MW = NKC * QG   # mask cols per qg (4096)
            for qg in range(NQG):
                qt_t[qg] = qpool.tile([128, QW], BF16, tag="qt", name=f"qt_{qg}")
                nc.sync.dma_start(qt_t[qg][:], qt_e[:, qg * QW:(qg + 1) * QW])
                mq_t[qg] = mpool.tile([128, MW], FP8, tag="mt", name=f"mq_{qg}")
                nc.sync.dma_start(mq_t[qg][:], mt_e[:, qg * MW:(qg + 1) * MW])

            ident_bf = cpool.tile([128, 128], BF16, tag="ident")
            make_identity(nc, ident_bf[:])

            def wsl(which, j):  # weight chunk slice in w_sb
                return w_sb[:, (which * NDM + j) * DK:(which * NDM + j + 1) * DK]

            # ---- head: k/v projections for this core's key slice ----
            kT_sb = cpool.tile([DK, SK], BF16, tag="kT")
            vT_sb = cpool.tile([DK, SK], BF16, tag="vT")
            for which, src, dst in ((1, kt_t, kT_sb), (2, vt_t, vT_sb)):
                ph = [pproj.tile([DK, QG], F32, tag="proj", name=f"pr{which}h{h}")
                      for h in range(2)]
                for j in range(NDM):
                    jp, jl = divmod(j, 2)
                    for h in range(2):
                        c0 = jl * SK + h * QG
                        nc.tensor.matmul(
                            ph[h][:], lhsT=wsl(which, j),
                            rhs=src[jp][:, c0:c0 + QG],
                            start=(j == 0), stop=(j == NDM - 1),
                        )
                for h in range(2):
                    nc.vector.tensor_scalar_add(
                        dst[:, h * QG:(h + 1) * QG], ph[h][:],
                        b_sb[:, which:which + 1],
                    )

            # ---- v_aug[128, 8*65]: col block kc = [vT_kc.T | ones] ----
            v_aug = cpool.tile([128, NKC * (DK + 1)], BF16, tag="vaug")
            nc.gpsimd.memset(v_aug[:], 1.0)
            for kc in range(NKC):
                pt = psT.tile([128, QG], BF16, tag="sT", name=f"vtr_{kc}")
                nc.tensor.transpose(
                    pt[:, :DK], vT_sb[:, kc * 128:(kc + 1) * 128],
                    ident_bf[:DK, :DK],
                )
                nc.vector.tensor_copy(
                    v_aug[:, kc * (DK + 1):kc * (DK + 1) + DK], pt[:, :DK]
                )

            # ---- main loop over query groups, software-pipelined ----
            qT = {}

            def issue_qproj(qg):
                ps = pproj.tile([DK, QG], F32, tag="proj", name=f"qps_{qg}")
                for j in range(NDM):
                    nc.tensor.matmul(
                        ps[:], lhsT=wsl(0, j),
                        rhs=qt_t[qg][:, j * QG:(j + 1) * QG],
                        start=(j == 0), stop=(j == NDM - 1),
                    )
                t = qtpool.tile([DK, QG], BF16, tag="qT", name=f"qT_{qg}")
                nc.vector.tensor_scalar_add(t[:], ps[:], b_sb[:, 0:1])
                qT[qg] = t

            issue_qproj(0)
            y_all = cpool.tile([DK + 1, S], F32, tag="y_all")
            for qg in range(NQG):
                if qg + 1 < NQG:
                    issue_qproj(qg + 1)
                yT = pyT.tile([DK + 1, QG], F32, tag="yT", name=f"yT_{qg}")
                p_t = []
                for kc in range(NKC):
                    sT = psT.tile([128, QG], F32, tag="sT", name=f"sT_{qg}_{kc}")
                    nc.tensor.matmul(
                        sT[:], lhsT=kT_sb[:, kc * 128:(kc + 1) * 128],
                        rhs=qT[qg][:], start=True, stop=True,
                    )
                    p = ppool.tile([128, QG], BF16, tag="p", name=f"p_{qg}_{kc}")
                    nc.scalar.activation(
                        p[:], sT[:], mybir.ActivationFunctionType.Exp, scale=0.125
                    )
                    eng = nc.vector if kc % 2 == 0 else nc.gpsimd
                    eng.tensor_mul(
                        p[:], p[:], mq_t[qg][:, kc * QG:(kc + 1) * QG]
                    )
                    p_t.append(p)
                for kc in range(NKC):
                    nc.tensor.matmul(
                        yT[:], lhsT=v_aug[:, kc * (DK + 1):(kc + 1) * (DK + 1)],
                        rhs=p_t[kc][:], start=(kc == 0), stop=(kc == NKC - 1),
                    )
                nc.scalar.copy(y_all[:, qg * QG:(qg + 1) * QG], yT[:])

            nc.sync.dma_start(out_e[:], y_all[:])

    nc.finalize()
    return nc


def _pack(at, w):
    """[R, W] -> [128, (R//128)*W]: row p gets rows {p, 128+p, ...}."""
    r = at.shape[0]
    return np.ascontiguousarray(
        at.reshape(r // 128, 128, w).transpose(1, 0, 2).reshape(128, -1)
    )


def kernel(Q, K, V, mask, Wq, bq, Wk, bk, Wv, bv):
    global _last_results
    bf16 = ml_dtypes.bfloat16
    fp8 = ml_dtypes.float8_e4m3

    w_p = np.concatenate(
        [_pack(W.T.astype(bf16), DK) for W in (Wq, Wk, Wv)], axis=1
    )
    b_p = np.ascontiguousarray(
        np.stack([bq, bk, bv], axis=1).astype(np.float32)
    )

    # qt per batch: [128, qg, j, 512]
    qt_b = []
    for b in range(B):
        a = Q[b].T.astype(bf16)                     # [1024 dm, 4096 q]
        a = a.reshape(NDM, 128, NQG, QG)            # [j, p, qg, q']
        qt_b.append(np.ascontiguousarray(
            a.transpose(1, 2, 0, 3).reshape(128, -1)
        ))

    in_maps = []
    for c in range(N_CORES):
        b, r = divmod(c, GROUP)
        rows = slice(r * SK, (r + 1) * SK)
        # mask tile: [128 s', qg, kc, 512 q']
        m = mask[b].T[rows, :].astype(fp8)          # [1024 s, 4096 q]
        m = m.reshape(NKC, 128, NQG, QG)            # [kc, p, qg, q']
        m = np.ascontiguousarray(m.transpose(1, 2, 0, 3).reshape(128, -1))
        in_maps.append({
            "kt": _pack(np.ascontiguousarray(K[b, rows, :].T).astype(bf16), SK),
            "vt": _pack(np.ascontiguousarray(V[b, rows, :].T).astype(bf16), SK),
            "qt": qt_b[b],
            "mt": m,
            "wqkv": w_p,
            "bqkv": b_p,
        })

    nc = _build()
    res = run_bass_kernel_spmd(nc, in_maps, core_ids=list(range(N_CORES)))
    _last_results = res

    out = np.empty((B, S, DK), dtype=np.float32)
    for b in range(B):
        acc = np.zeros((DK + 1, S), dtype=np.float32)
        for r in range(GROUP):
            acc += res.results[b * GROUP + r]["out"]
        out[b] = (acc[:DK, :] / acc[DK:DK + 1, :]).T
    return out


# revision 6
# speedup vs baseline: 1.3016x; 1.0665x over previous
"""Distributed Trainium2 (8 NeuronCores) attention-head kernel, key-sharded.

Problem: single attention head with projections.
  q = Q @ Wq.T + bq ; k = K @ Wk.T + bk ; v = V @ Wv.T + bv
  x = (q @ k.T) / sqrt(64) ; x = x*m - 1e9*(1-m) ; p = softmax(x)
  y = p @ v
Shapes: Q/K/V [2, 4096, 1024] f32, mask [2, 4096, 4096] int32 -> y [2, 4096, 64] f32.

Strategy (8 cores): shard KEYS 4-ways within each batch (core c -> batch c//4,
key rows (c%4)*1024..+1024).  Each core projects its 1024-key K/V slice (k/v
projection work is NOT duplicated, unlike query-sharding which replicates it
4x), projects all 4096 batch queries (q-proj is half the size of k+v), computes
partial attention p = exp(qk/8)*m over its key range, and returns the partial
numerator and denominator yT[65, 4096] = [v|1]^T @ p.  The host sums the 4
partials per batch and divides - algebraically identical to the reference's
masked softmax (no fully-masked rows exist).

Layouts ([partitions, free], contraction on partitions, all matmul N=512):
  head:   kT[64,1024] = sum_j WkT_j.T @ ktile_j   (PSUM, dm-chunk j)
          vT likewise; v_aug[128, 8*65] = [vT.T | 1] via PE transposes
  per qg (8 query-groups of 512):
          qT[64,512] = sum_j WqT_j.T @ qtile      (+bias on DVE)
          sT_kc[128,512] = kT_kc.T @ qT (8 kc) ; p[:, kc] = exp(sT/8) (Scalar)
          p *= mask (one DVE TENSOR_TENSOR per qg, [128,4096])
          yT[65,512] += v_aug_kc.T @ p_kc  (8 kc, PSUM accum)
          out[:, qg] <- yT  (DMA direct from PSUM, scalar HWDGE ring)
The y-matmuls of qg run one iteration later (software pipeline) so the PE
stream never waits on the exp->mask chain; scores(i) and y(i-1) interleave.
DMA issue order == consumption order (per-engine FIFO queues preserve it).
"""

import numpy as np
import ml_dtypes

import concourse.bass as bass
import concourse.mybir as mybir
import concourse.tile as tile
from concourse import bacc
import concourse.bass_utils as bass_utils
from concourse.bass_utils import run_bass_kernel_spmd
from concourse.masks import make_identity

B, S, DM, DK = 2, 4096, 1024, 64
N_CORES = 8
GROUP = 4              # cores per batch
SK = S // GROUP        # key rows per core (1024)
NDM = DM // 128        # dm chunks (8)
NQG = 8                # query groups
QG = S // NQG          # 512 queries per group
NKC = SK // 128        # key chunks per core (8)

F32 = mybir.dt.float32
BF16 = mybir.dt.bfloat16
FP8 = mybir.dt.float8e4

_last_results = None


def _build():
    nc = bacc.Bacc(None, target_bir_lowering=False)

    # kt/vt: [128, j(8) x 1024 keys] bf16; qt: [128, qg(8) x j(8) x 512] bf16
    # mt: [128, qg(8) x kc(8) x 512] fp8; w: [128, 3 x 8 x 64] bf16
    kt_e = nc.declare_dram_parameter("kt", [128, NDM * SK], BF16, isOutput=False)
    vt_e = nc.declare_dram_parameter("vt", [128, NDM * SK], BF16, isOutput=False)
    qt_e = nc.declare_dram_parameter("qt", [128, NQG * NDM * QG], BF16, isOutput=False)
    mt_e = nc.declare_dram_parameter("mt", [128, NQG * NKC * QG], FP8, isOutput=False)
    w_e = nc.declare_dram_parameter("wqkv", [128, 3 * NDM * DK], BF16, isOutput=False)
    b_e = nc.declare_dram_parameter("bqkv", [DK, 3], F32, isOutput=False)
    out_e = nc.declare_dram_parameter("out", [DK + 1, S], F32, isOutput=True)

    QW = NDM * QG   # qt cols per qg (4096)
    MW = NKC * QG   # mask cols per qg (4096)

    with tile.TileContext(nc) as tc:
        with (
            tc.tile_pool(name="const", bufs=1) as cpool,
            tc.tile_pool(name="kvin", bufs=8) as kvpool,
            tc.tile_pool(name="qin", bufs=3) as qpool,
            tc.tile_pool(name="min", bufs=3) as mpool,
            tc.tile_pool(name="qT", bufs=3) as qtpool,
            tc.tile_pool(name="p", bufs=3) as ppool,
            tc.tile_pool(name="psum_pr", bufs=2, space="PSUM") as pproj,
            tc.tile_pool(name="psum_s", bufs=4, space="PSUM") as psT,
            tc.tile_pool(name="psum_y", bufs=2, space="PSUM") as pyT,
        ):
            # ---- DMA loads (sync HWDGE ring), in consumption order ----
            w_sb = cpool.tile([128, 3 * NDM * DK], BF16, tag="w")
            nc.sync.dma_start(w_sb[:], w_e[:])
            b_sb = cpool.tile([DK, 3], F32, tag="b")
            nc.sync.dma_start(b_sb[:], b_e[:])

            qt_t, mq_t = {}, {}

            def load_q(qg):
                qt_t[qg] = qpool.tile([128, QW], BF16, tag="qt", name=f"qt_{qg}")
                nc.sync.dma_start(qt_t[qg][:], qt_e[:, qg * QW:(qg + 1) * QW])

            def load_m(qg):
                mq_t[qg] = mpool.tile([128, MW], FP8, tag="mt", name=f"mq_{qg}")
                nc.sync.dma_start(mq_t[qg][:], mt_e[:, qg * MW:(qg + 1) * MW])

            load_q(0)
            kt_t, vt_t = [], []
            for jp in range(4):
                t = kvpool.tile([128, 2 * SK], BF16, tag="kv", name=f"kt_{jp}")
                nc.sync.dma_start(t[:], kt_e[:, jp * 2 * SK:(jp + 1) * 2 * SK])
                kt_t.append(t)
            load_m(0)
            for jp in range(4):
                t = kvpool.tile([128, 2 * SK], BF16, tag="kv", name=f"vt_{jp}")
                nc.sync.dma_start(t[:], vt_e[:, jp * 2 * SK:(jp + 1) * 2 * SK])
                vt_t.append(t)
            for qg in range(1, NQG):
                load_q(qg)
                load_m(qg)

            ident_bf = cpool.tile([128, 128], BF16, tag="ident")
            make_identity(nc, ident_bf[:])

            def wsl(which, j):  # weight chunk slice in w_sb
                return w_sb[:, (which * NDM + j) * DK:(which * NDM + j + 1) * DK]

            def proj_kv(which, src, dst):
                ph = [pproj.tile([DK, QG], F32, tag="proj", name=f"pr{which}h{h}")
                      for h in range(2)]
                for j in range(NDM):
                    jp, jl = divmod(j, 2)
                    for h in range(2):
                        c0 = jl * SK + h * QG
                        nc.tensor.matmul(
                            ph[h][:], lhsT=wsl(which, j),
                            rhs=src[jp][:, c0:c0 + QG],
                            start=(j == 0), stop=(j == NDM - 1),
                        )
                for h in range(2):
                    nc.vector.tensor_scalar_add(
                        dst[:, h * QG:(h + 1) * QG], ph[h][:],
                        b_sb[:, which:which + 1],
                    )

            kT_sb = cpool.tile([DK, SK], BF16, tag="kT")
            vT_sb = cpool.tile([DK, SK], BF16, tag="vT")
            v_aug = cpool.tile([128, NKC * (DK + 1)], BF16, tag="vaug")
            nc.gpsimd.memset(v_aug[:], 1.0)

            qT = {}

            def issue_qproj(qg):
                ps = pproj.tile([DK, QG], F32, tag="proj", name=f"qps_{qg}")
                for j in range(NDM):
                    nc.tensor.matmul(
                        ps[:], lhsT=wsl(0, j),
                        rhs=qt_t[qg][:, j * QG:(j + 1) * QG],
                        start=(j == 0), stop=(j == NDM - 1),
                    )
                t = qtpool.tile([DK, QG], BF16, tag="qT", name=f"qT_{qg}")
                nc.vector.tensor_scalar_add(t[:], ps[:], b_sb[:, 0:1])
                qT[qg] = t

            # ---- head: q-proj(0) first (qt0 is the first DMA), then k-proj ----
            issue_qproj(0)
            proj_kv(1, kt_t, kT_sb)

            # ---- main loop, software-pipelined: scores(i) + y(i-1) ----
            p_t, yT_t = {}, {}

            def scores_half(i, half):
                for kc in range(half * 4, half * 4 + 4):
                    sT = psT.tile([128, QG], F32, tag="sT", name=f"sT_{i}_{kc}")
                    nc.tensor.matmul(
                        sT[:], lhsT=kT_sb[:, kc * 128:(kc + 1) * 128],
                        rhs=qT[i][:], start=True, stop=True,
                    )
                    nc.scalar.activation(
                        p_t[i][:, kc * QG:(kc + 1) * QG], sT[:],
                        mybir.ActivationFunctionType.Exp, scale=0.125,
                    )

            def y_half(i, half):
                for kc in range(half * 4, half * 4 + 4):
                    nc.tensor.matmul(
                        yT_t[i][:],
                        lhsT=v_aug[:, kc * (DK + 1):(kc + 1) * (DK + 1)],
                        rhs=p_t[i][:, kc * QG:(kc + 1) * QG],
                        start=(kc == 0), stop=(kc == NKC - 1),
                    )

            y_all = cpool.tile([DK + 1, S], F32, tag="y_all")

            def store(i):
                nc.scalar.copy(y_all[:, i * QG:(i + 1) * QG], yT_t[i][:])
                nc.scalar.dma_start(
                    out_e[:, i * QG:(i + 1) * QG],
                    y_all[:, i * QG:(i + 1) * QG],
                )

            for i in range(NQG):
                p_t[i] = ppool.tile([128, NKC * QG], BF16, tag="p", name=f"p_{i}")
                yT_t[i] = pyT.tile([DK + 1, QG], F32, tag="yT", name=f"yT_{i}")
                if i + 1 < NQG:
                    issue_qproj(i + 1)
                scores_half(i, 0)
                if i >= 1:
                    y_half(i - 1, 0)
                scores_half(i, 1)
                if i >= 1:
                    y_half(i - 1, 1)
                    store(i - 1)
                if i == 0:
                    # v-proj + v_aug build, issued after scores(0) so the PE
                    # isn't blocked waiting on the vt DMAs
                    proj_kv(2, vt_t, vT_sb)
                    for kc in range(NKC):
                        pt = psT.tile([128, QG], BF16, tag="sT", name=f"vtr_{kc}")
                        nc.tensor.transpose(
                            pt[:, :DK], vT_sb[:, kc * 128:(kc + 1) * 128],
                            ident_bf[:DK, :DK],
                        )
                        nc.vector.tensor_copy(
                            v_aug[:, kc * (DK + 1):kc * (DK + 1) + DK], pt[:, :DK]
                        )
                # mask multiply: one big TT per qg (split in halves on the
                # last qg to shorten the tail)
                if i == NQG - 1:
                    for h in range(2):
                        nc.vector.tensor_mul(
                            p_t[i][:, h * 4 * QG:(h + 1) * 4 * QG],
                            p_t[i][:, h * 4 * QG:(h + 1) * 4 * QG],
                            mq_t[i][:, h * 4 * QG:(h + 1) * 4 * QG],
                        )
                else:
                    nc.vector.tensor_mul(p_t[i][:], p_t[i][:], mq_t[i][:])

            y_half(NQG - 1, 0)
            y_half(NQG - 1, 1)
            store(NQG - 1)

    nc.finalize()
    return nc


def _pack(at, w):
    """[R, W] -> [128, (R//128)*W]: row p gets rows {p, 128+p, ...}."""
    r = at.shape[0]
    return np.ascontiguousarray(
        at.reshape(r // 128, 128, w).transpose(1, 0, 2).reshape(128, -1)
    )


def kernel(Q, K, V, mask, Wq, bq, Wk, bk, Wv, bv):
    global _last_results
    bf16 = ml_dtypes.bfloat16
    fp8 = ml_dtypes.float8_e4m3

    w_p = np.concatenate(
        [_pack(W.T.astype(bf16), DK) for W in (Wq, Wk, Wv)], axis=1
    )
    b_p = np.ascontiguousarray(
        np.stack([bq, bk, bv], axis=1).astype(np.float32)
    )

    # qt per batch: [128, qg, j, 512]
    qt_b = []
    for b in range(B):
        a = Q[b].T.astype(bf16)                     # [1024 dm, 4096 q]
        a = a.reshape(NDM, 128, NQG, QG)            # [j, p, qg, q']
        qt_b.append(np.ascontiguousarray(
            a.transpose(1, 2, 0, 3).reshape(128, -1)
        ))

    in_maps = []
    for c in range(N_CORES):
        b, r = divmod(c, GROUP)
        rows = slice(r * SK, (r + 1) * SK)
        # mask tile: [128 s', qg, kc, 512 q']
        m = mask[b].T[rows, :].astype(fp8)          # [1024 s, 4096 q]
        m = m.reshape(NKC, 128, NQG, QG)            # [kc, p, qg, q']
        m = np.ascontiguousarray(m.transpose(1, 2, 0, 3).reshape(128, -1))
        in_maps.append({
            "kt": _pack(np.ascontiguousarray(K[b, rows, :].T).astype(bf16), SK),
            "vt": _pack(np.ascontiguousarray(V[b, rows, :].T).astype(bf16), SK),
            "qt": qt_b[b],
            "mt": m,
            "wqkv": w_p,
            "bqkv": b_p,
        })

    nc = _build()
    res = run_bass_kernel_spmd(nc, in_maps, core_ids=list(range(N_CORES)))
    _last_results = res

    out = np.empty((B, S, DK), dtype=np.float32)
    for b in range(B):
        acc = np.zeros((DK + 1, S), dtype=np.float32)
        for r in range(GROUP):
            acc += res.results[b * GROUP + r]["out"]
        out[b] = (acc[:DK, :] / acc[DK:DK + 1, :]).T
    return out
